# revision 5
# baseline (speedup 1.0000x reference)
"""GRU cell kernel for Trainium2, data-parallel over 8 NeuronCores.

Math (per reference):
    z = sigmoid(x @ wz.T + h @ uz.T + bz)
    r = sigmoid(x @ wr.T + h @ ur.T + br)
    g = tanh(x @ wh.T + (r*h) @ uh.T + bh)
    out = (1-z)*h + z*g = h + z*(g - h)

Everything on-device is computed in TRANSPOSED layout ([feature, row]),
so that both matmul operands arrive with the contraction dim on
partitions without any on-device transpose:
    outT = f(W_T_block.T @ xT)  with W_T = W.T ([in, out]) prepped on host.
The host transposes x/h/W on the way in and the output on the way out.

Sharding: rows 16384 -> 8 cores x 2048 rows, weights replicated.
"""

import numpy as np
import ml_dtypes
from contextlib import ExitStack

import concourse.bass as bass
import concourse.bacc as bacc
import concourse.mybir as mybir
import concourse.tile as tile
from concourse.bass_utils import run_bass_kernel_spmd

H = 1024
N_ROWS = 16384
NCORES = 8
P = 128
KB = H // P            # 8 contraction blocks
MB = H // P            # 8 output-feature blocks
NS = 512               # rows per matmul moving slice (one PSUM bank)

BF = mybir.dt.bfloat16
F32 = mybir.dt.float32
AF = mybir.ActivationFunctionType
bf16 = ml_dtypes.bfloat16

# Set by test harness to capture a trace; harness-facing default off.
TRACE = False
LAST_RESULT = None


def build_nc(R=N_ROWS // NCORES, CH=2):
    """Build the per-core Bass program. R rows per core, CH row-chunks."""
    RC = R // CH           # rows per chunk
    SL = RC // NS          # moving slices per chunk

    nc = bacc.Bacc(trn_type="TRN2", target_bir_lowering=False,
                   debug=False, enable_asserts=False)

    xT = nc.dram_tensor("xT", [H, R], BF, kind="ExternalInput").ap()
    hTb = nc.dram_tensor("hTb", [H, R], BF, kind="ExternalInput").ap()
    hTf = nc.dram_tensor("hTf", [H, R], F32, kind="ExternalInput").ap()
    wd = {
        nm: nc.dram_tensor(nm, [H, H], BF, kind="ExternalInput").ap()
        for nm in ("wzT", "uzT", "wrT", "urT", "whT", "uhT")
    }
    bias = nc.dram_tensor("bias", [P, 3 * MB], F32, kind="ExternalInput").ap()
    outT = nc.dram_tensor("outT", [H, R], F32, kind="ExternalOutput").ap()

    with tile.TileContext(nc) as tc, ExitStack() as ctx:
        wpool = ctx.enter_context(tc.tile_pool(name="w", bufs=32))
        xpool = ctx.enter_context(tc.tile_pool(name="x", bufs=2))
        hbpool = ctx.enter_context(tc.tile_pool(name="hb", bufs=1))
        hfpool = ctx.enter_context(tc.tile_pool(name="hf", bufs=2))
        rhpool = ctx.enter_context(tc.tile_pool(name="rh", bufs=MB + 2))
        rpool = ctx.enter_context(tc.tile_pool(name="r", bufs=6))
        zpool = ctx.enter_context(tc.tile_pool(name="z", bufs=2 * MB + 2))
        gpool = ctx.enter_context(tc.tile_pool(name="g", bufs=6))
        opool = ctx.enter_context(tc.tile_pool(name="o", bufs=2))
        cpool = ctx.enter_context(tc.tile_pool(name="c", bufs=1))
        pspool = ctx.enter_context(tc.tile_pool(name="ps", bufs=8, space="PSUM"))

        # Warm up the ACT table set (sigmoid_and_others covers tanh too) on an
        # instruction with minimal sync waits — walrus can't attach the
        # PSEUDO_LOAD_ACT_FUNC_SET to an activation that already carries two
        # sem waits ("Too many sync wait commands").
        warm = cpool.tile([P, 8], F32, tag="warm")
        nc.gpsimd.memset(warm[:], 0.0)
        nc.scalar.activation(warm[:], warm[:], AF.Sigmoid)

        bt = cpool.tile([P, 3 * MB], F32, tag="bias")
        nc.sync.dma_start(bt[:], bias[:])
        # bias column layout: [z:0..7 | r:8..15 | h:16..23]
        GZ, GR, GH = 0, 1, 2

        def load_w(name, c):
            """8 k-tiles [P, H] of one weight matrix."""
            ts = []
            for k in range(KB):
                t = wpool.tile([P, H], BF, tag="w")
                nc.sync.dma_start(t[:], wd[name][k * P:(k + 1) * P, :])
                ts.append(t)
            return ts

        def mm_group(psums, wt, ut, mov_w, mov_u, m, c):
            """Accumulate  wt.T @ mov_w + ut.T @ mov_u  for feature block m
            into psums[s] ([P, NS] each), contracting over all KB blocks."""
            msl = slice(m * P, (m + 1) * P)
            for k in range(KB):
                for s in range(SL):
                    nc.tensor.matmul(
                        psums[s][:],
                        wt[k][:, msl],
                        mov_w[:, k * RC + s * NS: k * RC + (s + 1) * NS],
                        start=(k == 0), stop=False,
                    )
            for k in range(KB):
                for s in range(SL):
                    nc.tensor.matmul(
                        psums[s][:],
                        ut[k][:, msl],
                        mov_u[:, k * RC + s * NS: k * RC + (s + 1) * NS],
                        start=False, stop=(k == KB - 1),
                    )

        for c in range(CH):
            rows = slice(c * RC, (c + 1) * RC)

            xt = xpool.tile([P, KB * RC], BF, tag="x")
            hbt = hbpool.tile([P, KB * RC], BF, tag="hb")
            for k in range(KB):
                ksl = slice(k * P, (k + 1) * P)
                nc.sync.dma_start(xt[:, k * RC:(k + 1) * RC], xT[ksl, rows])
                nc.sync.dma_start(hbt[:, k * RC:(k + 1) * RC], hTb[ksl, rows])

            # ---- r pass ----
            wr = load_w("wrT", c)
            ur = load_w("urT", c)
            rhs = []
            for m in range(MB):
                ps = [pspool.tile([P, NS], F32, tag="ps", name="ps") for _ in range(SL)]
                mm_group(ps, wr, ur, xt, hbt, m, c)
                rh = rhpool.tile([P, RC], BF, tag="rh")
                for s in range(SL):
                    rt = rpool.tile([P, NS], BF, tag="r")
                    nc.scalar.activation(rt[:], ps[s][:], AF.Sigmoid,
                                         bias=bt[:, GR * MB + m: GR * MB + m + 1])
                    nc.vector.tensor_mul(
                        rh[:, s * NS:(s + 1) * NS], rt[:],
                        hbt[:, m * RC + s * NS: m * RC + (s + 1) * NS])
                rhs.append(rh)

            # ---- z pass ----
            wz = load_w("wzT", c)
            uz = load_w("uzT", c)
            zts = []
            for m in range(MB):
                ps = [pspool.tile([P, NS], F32, tag="ps", name="ps") for _ in range(SL)]
                mm_group(ps, wz, uz, xt, hbt, m, c)
                zm = []
                for s in range(SL):
                    zt = zpool.tile([P, NS], BF, tag="z")
                    nc.scalar.activation(zt[:], ps[s][:], AF.Sigmoid,
                                         bias=bt[:, GZ * MB + m: GZ * MB + m + 1])
                    zm.append(zt)
                zts.append(zm)

            # ---- h~ pass + combine ----
            wh = load_w("whT", c)
            uh = load_w("uhT", c)
            for m in range(MB):
                msl = slice(m * P, (m + 1) * P)
                ps = [pspool.tile([P, NS], F32, tag="ps", name="ps") for _ in range(SL)]
                for k in range(KB):
                    for s in range(SL):
                        nc.tensor.matmul(
                            ps[s][:], wh[k][:, msl],
                            xt[:, k * RC + s * NS: k * RC + (s + 1) * NS],
                            start=(k == 0), stop=False)
                for k in range(KB):
                    for s in range(SL):
                        nc.tensor.matmul(
                            ps[s][:], uh[k][:, msl],
                            rhs[k][:, s * NS:(s + 1) * NS],
                            start=False, stop=(k == KB - 1))
                hf = hfpool.tile([P, RC], F32, tag="hf")
                nc.sync.dma_start(hf[:], hTf[msl, rows])
                ot = opool.tile([P, RC], F32, tag="o")
                for s in range(SL):
                    ssl = slice(s * NS, (s + 1) * NS)
                    gt = gpool.tile([P, NS], F32, tag="g")
                    nc.scalar.activation(gt[:], ps[s][:], AF.Tanh,
                                         bias=bt[:, GH * MB + m: GH * MB + m + 1])
                    # g-h ; z*(g-h) ; h + z*(g-h)
                    nc.vector.tensor_sub(gt[:], gt[:], hf[:, ssl])
                    nc.vector.tensor_mul(gt[:], zts[m][s][:], gt[:])
                    nc.vector.tensor_add(ot[:, ssl], gt[:], hf[:, ssl])
                nc.sync.dma_start(outT[msl, rows], ot[:])

    nc.compile()
    return nc


_NC_CACHE = {}


def _get_nc(R, CH):
    key = (R, CH)
    if key not in _NC_CACHE:
        _NC_CACHE[key] = build_nc(R, CH)
    return _NC_CACHE[key]


def make_in_maps(update, hidden, wz, uz, bz, wr, ur, br, wh, uh, bh,
                 ncores=NCORES):
    wmap = {
        "wzT": np.ascontiguousarray(wz.T).astype(bf16),
        "uzT": np.ascontiguousarray(uz.T).astype(bf16),
        "wrT": np.ascontiguousarray(wr.T).astype(bf16),
        "urT": np.ascontiguousarray(ur.T).astype(bf16),
        "whT": np.ascontiguousarray(wh.T).astype(bf16),
        "uhT": np.ascontiguousarray(uh.T).astype(bf16),
    }
    bias = np.empty((P, 3 * MB), np.float32)
    for g, b in enumerate((bz, br, bh)):
        bias[:, g * MB:(g + 1) * MB] = np.asarray(b, np.float32).reshape(MB, P).T
    rows = update.shape[0]
    rc = rows // ncores
    in_maps = []
    for i in range(ncores):
        sl = slice(i * rc, (i + 1) * rc)
        xTs = np.ascontiguousarray(np.asarray(update[sl], np.float32).T)
        hTs = np.ascontiguousarray(np.asarray(hidden[sl], np.float32).T)
        in_maps.append(dict(xT=xTs.astype(bf16), hTb=hTs.astype(bf16),
                            hTf=hTs, bias=bias, **wmap))
    return in_maps


def kernel(update, hidden, wz, uz, bz, wr, ur, br, wh, uh, bh):
    global LAST_RESULT
    update = np.asarray(update)
    hidden = np.asarray(hidden)
    R = update.shape[0] // NCORES
    nc = _get_nc(R, 2)
    in_maps = make_in_maps(update, hidden, wz, uz, bz, wr, ur, br, wh, uh, bh)
    res = run_bass_kernel_spmd(nc, in_maps, list(range(NCORES)), trace=TRACE)
    LAST_RESULT = res
    out = np.empty((update.shape[0], H), np.float32)
    for i in range(NCORES):
        out[i * R:(i + 1) * R] = res.results[i]["outT"].T
    return out


# revision 8
# speedup vs baseline: 1.0037x; 1.0037x over previous
"""GRU cell kernel for Trainium2, data-parallel over 8 NeuronCores.

Math (per reference):
    z = sigmoid(x @ wz.T + h @ uz.T + bz)
    r = sigmoid(x @ wr.T + h @ ur.T + br)
    g = tanh(x @ wh.T + (r*h) @ uh.T + bh)
    out = (1-z)*h + z*g = h + z*(g - h)

Everything on-device is computed in TRANSPOSED layout ([feature, row]),
so that both matmul operands arrive with the contraction dim on
partitions without any on-device transpose:
    outT = f(W_T_block.T @ xT)  with W_T = W.T ([in, out]) prepped on host.
The host transposes x/h/W on the way in and the output on the way out.

Sharding: rows 16384 -> 8 cores x 2048 rows, weights replicated.
"""

import numpy as np
import ml_dtypes
from contextlib import ExitStack

import concourse.bass as bass
import concourse.bacc as bacc
import concourse.mybir as mybir
import concourse.tile as tile
from concourse.bass_utils import run_bass_kernel_spmd

H = 1024
N_ROWS = 16384
NCORES = 8
P = 128
KB = H // P            # 8 contraction blocks
MB = H // P            # 8 output-feature blocks
NS = 512               # rows per matmul moving slice (one PSUM bank)

BF = mybir.dt.bfloat16
F32 = mybir.dt.float32
AF = mybir.ActivationFunctionType
bf16 = ml_dtypes.bfloat16

# Set by test harness to capture a trace; harness-facing default off.
TRACE = False
LAST_RESULT = None


def build_nc(R=N_ROWS // NCORES, CH=2):
    """Build the per-core Bass program. R rows per core, CH row-chunks."""
    RC = R // CH           # rows per chunk
    SL = RC // NS          # moving slices per chunk

    nc = bacc.Bacc(trn_type="TRN2", target_bir_lowering=False,
                   debug=False, enable_asserts=False)

    xT = nc.dram_tensor("xT", [H, R], BF, kind="ExternalInput").ap()
    hTb = nc.dram_tensor("hTb", [H, R], BF, kind="ExternalInput").ap()
    hTf = nc.dram_tensor("hTf", [H, R], F32, kind="ExternalInput").ap()
    wd = {
        nm: nc.dram_tensor(nm, [H, H], BF, kind="ExternalInput").ap()
        for nm in ("wzT", "uzT", "wrT", "urT", "whT", "uhT")
    }
    bias = nc.dram_tensor("bias", [P, 3 * MB], F32, kind="ExternalInput").ap()
    outT = nc.dram_tensor("outT", [H, R], F32, kind="ExternalOutput").ap()

    with tile.TileContext(nc) as tc, ExitStack() as ctx:
        wpool = ctx.enter_context(tc.tile_pool(name="w", bufs=32))
        xpool = ctx.enter_context(tc.tile_pool(name="x", bufs=2))
        hbpool = ctx.enter_context(tc.tile_pool(name="hb", bufs=1))
        hfpool = ctx.enter_context(tc.tile_pool(name="hf", bufs=2))
        rhpool = ctx.enter_context(tc.tile_pool(name="rh", bufs=MB + 2))
        rpool = ctx.enter_context(tc.tile_pool(name="r", bufs=6))
        zpool = ctx.enter_context(tc.tile_pool(name="z", bufs=2 * MB + 2))
        gpool = ctx.enter_context(tc.tile_pool(name="g", bufs=6))
        opool = ctx.enter_context(tc.tile_pool(name="o", bufs=2))
        cpool = ctx.enter_context(tc.tile_pool(name="c", bufs=1))
        pspool = ctx.enter_context(tc.tile_pool(name="ps", bufs=8, space="PSUM"))

        # Warm up the ACT table set (sigmoid_and_others covers tanh too) on an
        # instruction with minimal sync waits — walrus can't attach the
        # PSEUDO_LOAD_ACT_FUNC_SET to an activation that already carries two
        # sem waits ("Too many sync wait commands").
        warm = cpool.tile([P, 8], F32, tag="warm")
        nc.gpsimd.memset(warm[:], 0.0)
        nc.scalar.activation(warm[:], warm[:], AF.Sigmoid)

        bt = cpool.tile([P, 3 * MB], F32, tag="bias")
        nc.sync.dma_start(bt[:], bias[:])
        # bias column layout: [z:0..7 | r:8..15 | h:16..23]
        GZ, GR, GH = 0, 1, 2

        def load_w(name, c):
            """8 k-tiles [P, H] of one weight matrix."""
            ts = []
            for k in range(KB):
                t = wpool.tile([P, H], BF, tag="w")
                nc.sync.dma_start(t[:], wd[name][k * P:(k + 1) * P, :])
                ts.append(t)
            return ts

        def mm_group(psums, wt, ut, mov_w, mov_u, m, c):
            """Accumulate  wt.T @ mov_w + ut.T @ mov_u  for feature block m
            into psums[s] ([P, NS] each), contracting over all KB blocks."""
            msl = slice(m * P, (m + 1) * P)
            for k in range(KB):
                for s in range(SL):
                    nc.tensor.matmul(
                        psums[s][:],
                        wt[k][:, msl],
                        mov_w[:, k * RC + s * NS: k * RC + (s + 1) * NS],
                        start=(k == 0), stop=False,
                    )
            for k in range(KB):
                for s in range(SL):
                    nc.tensor.matmul(
                        psums[s][:],
                        ut[k][:, msl],
                        mov_u[:, k * RC + s * NS: k * RC + (s + 1) * NS],
                        start=False, stop=(k == KB - 1),
                    )

        for c in range(CH):
            rows = slice(c * RC, (c + 1) * RC)

            # r-pass weights BEFORE x/h: the first matmul's weight tile must
            # not queue behind 4MB of activation DMA (measured ~12us of PE
            # idle at kernel start from exactly that).
            wr = load_w("wrT", c)
            ur = load_w("urT", c)

            xt = xpool.tile([P, KB * RC], BF, tag="x")
            hbt = hbpool.tile([P, KB * RC], BF, tag="hb")
            for k in range(KB):
                ksl = slice(k * P, (k + 1) * P)
                nc.sync.dma_start(xt[:, k * RC:(k + 1) * RC], xT[ksl, rows])
                nc.sync.dma_start(hbt[:, k * RC:(k + 1) * RC], hTb[ksl, rows])

            # ---- r pass ----
            rhs = []
            for m in range(MB):
                ps = [pspool.tile([P, NS], F32, tag="ps", name="ps") for _ in range(SL)]
                mm_group(ps, wr, ur, xt, hbt, m, c)
                rh = rhpool.tile([P, RC], BF, tag="rh")
                for s in range(SL):
                    rt = rpool.tile([P, NS], BF, tag="r")
                    nc.scalar.activation(rt[:], ps[s][:], AF.Sigmoid,
                                         bias=bt[:, GR * MB + m: GR * MB + m + 1])
                    nc.vector.tensor_mul(
                        rh[:, s * NS:(s + 1) * NS], rt[:],
                        hbt[:, m * RC + s * NS: m * RC + (s + 1) * NS])
                rhs.append(rh)

            # ---- z pass ----
            wz = load_w("wzT", c)
            uz = load_w("uzT", c)
            zts = []
            for m in range(MB):
                ps = [pspool.tile([P, NS], F32, tag="ps", name="ps") for _ in range(SL)]
                mm_group(ps, wz, uz, xt, hbt, m, c)
                zm = []
                for s in range(SL):
                    zt = zpool.tile([P, NS], BF, tag="z")
                    nc.scalar.activation(zt[:], ps[s][:], AF.Sigmoid,
                                         bias=bt[:, GZ * MB + m: GZ * MB + m + 1])
                    zm.append(zt)
                zts.append(zm)

            # ---- h~ pass + combine ----
            wh = load_w("whT", c)
            uh = load_w("uhT", c)
            for m in range(MB):
                msl = slice(m * P, (m + 1) * P)
                hf = hfpool.tile([P, RC], F32, tag="hf")
                nc.sync.dma_start(hf[:], hTf[msl, rows])
                ps = [pspool.tile([P, NS], F32, tag="ps", name="ps") for _ in range(SL)]
                for k in range(KB):
                    for s in range(SL):
                        nc.tensor.matmul(
                            ps[s][:], wh[k][:, msl],
                            xt[:, k * RC + s * NS: k * RC + (s + 1) * NS],
                            start=(k == 0), stop=False)
                for k in range(KB):
                    for s in range(SL):
                        nc.tensor.matmul(
                            ps[s][:], uh[k][:, msl],
                            rhs[k][:, s * NS:(s + 1) * NS],
                            start=False, stop=(k == KB - 1))
                ot = opool.tile([P, RC], F32, tag="o")
                for s in range(SL):
                    ssl = slice(s * NS, (s + 1) * NS)
                    gt = gpool.tile([P, NS], F32, tag="g")
                    nc.scalar.activation(gt[:], ps[s][:], AF.Tanh,
                                         bias=bt[:, GH * MB + m: GH * MB + m + 1])
                    # g-h ; z*(g-h) ; h + z*(g-h)
                    nc.vector.tensor_sub(gt[:], gt[:], hf[:, ssl])
                    nc.vector.tensor_mul(gt[:], zts[m][s][:], gt[:])
                    nc.vector.tensor_add(ot[:, ssl], gt[:], hf[:, ssl])
                    # per-slice store so the tail DMA streams out as each
                    # slice's combine finishes instead of all at once
                    nc.sync.dma_start(
                        outT[msl, c * RC + s * NS: c * RC + (s + 1) * NS],
                        ot[:, ssl])

    nc.compile()
    return nc


_NC_CACHE = {}


def _get_nc(R, CH):
    key = (R, CH)
    if key not in _NC_CACHE:
        _NC_CACHE[key] = build_nc(R, CH)
    return _NC_CACHE[key]


def make_in_maps(update, hidden, wz, uz, bz, wr, ur, br, wh, uh, bh,
                 ncores=NCORES):
    wmap = {
        "wzT": np.ascontiguousarray(wz.T).astype(bf16),
        "uzT": np.ascontiguousarray(uz.T).astype(bf16),
        "wrT": np.ascontiguousarray(wr.T).astype(bf16),
        "urT": np.ascontiguousarray(ur.T).astype(bf16),
        "whT": np.ascontiguousarray(wh.T).astype(bf16),
        "uhT": np.ascontiguousarray(uh.T).astype(bf16),
    }
    bias = np.empty((P, 3 * MB), np.float32)
    for g, b in enumerate((bz, br, bh)):
        bias[:, g * MB:(g + 1) * MB] = np.asarray(b, np.float32).reshape(MB, P).T
    rows = update.shape[0]
    rc = rows // ncores
    in_maps = []
    for i in range(ncores):
        sl = slice(i * rc, (i + 1) * rc)
        xTs = np.ascontiguousarray(np.asarray(update[sl], np.float32).T)
        hTs = np.ascontiguousarray(np.asarray(hidden[sl], np.float32).T)
        in_maps.append(dict(xT=xTs.astype(bf16), hTb=hTs.astype(bf16),
                            hTf=hTs, bias=bias, **wmap))
    return in_maps


def kernel(update, hidden, wz, uz, bz, wr, ur, br, wh, uh, bh):
    global LAST_RESULT
    update = np.asarray(update)
    hidden = np.asarray(hidden)
    R = update.shape[0] // NCORES
    nc = _get_nc(R, 2)
    in_maps = make_in_maps(update, hidden, wz, uz, bz, wr, ur, br, wh, uh, bh)
    res = run_bass_kernel_spmd(nc, in_maps, list(range(NCORES)), trace=TRACE)
    LAST_RESULT = res
    out = np.empty((update.shape[0], H), np.float32)
    for i in range(NCORES):
        out[i * R:(i + 1) * R] = res.results[i]["outT"].T
    return out


# revision 9
# speedup vs baseline: 1.0313x; 1.0275x over previous
"""GRU cell kernel for Trainium2, data-parallel over 8 NeuronCores.

Math (per reference):
    z = sigmoid(x @ wz.T + h @ uz.T + bz)
    r = sigmoid(x @ wr.T + h @ ur.T + br)
    g = tanh(x @ wh.T + (r*h) @ uh.T + bh)
    out = (1-z)*h + z*g = h + z*(g - h)

Everything on-device is computed in TRANSPOSED layout ([feature, row]),
so that both matmul operands arrive with the contraction dim on
partitions without any on-device transpose:
    outT = f(W_T_block.T @ xT)  with W_T = W.T ([in, out]) prepped on host.
The host transposes x/h/W on the way in and the output on the way out.

Sharding: rows 16384 -> 8 cores x 2048 rows, weights replicated.
"""

import numpy as np
import ml_dtypes
from contextlib import ExitStack

import concourse.bass as bass
import concourse.bacc as bacc
import concourse.mybir as mybir
import concourse.tile as tile
from concourse.bass_utils import run_bass_kernel_spmd

H = 1024
N_ROWS = 16384
NCORES = 8
P = 128
KB = H // P            # 8 contraction blocks
MB = H // P            # 8 output-feature blocks
NS = 512               # rows per matmul moving slice (one PSUM bank)

BF = mybir.dt.bfloat16
F32 = mybir.dt.float32
AF = mybir.ActivationFunctionType
bf16 = ml_dtypes.bfloat16

# Set by test harness to capture a trace; harness-facing default off.
TRACE = False
LAST_RESULT = None


def build_nc(R=N_ROWS // NCORES, CH=2):
    """Build the per-core Bass program. R rows per core, CH row-chunks."""
    RC = R // CH           # rows per chunk
    SL = RC // NS          # moving slices per chunk

    nc = bacc.Bacc(trn_type="TRN2", target_bir_lowering=False,
                   debug=False, enable_asserts=False)

    xT = nc.dram_tensor("xT", [H, R], BF, kind="ExternalInput").ap()
    hTb = nc.dram_tensor("hTb", [H, R], BF, kind="ExternalInput").ap()
    hTf = nc.dram_tensor("hTf", [H, R], F32, kind="ExternalInput").ap()
    wd = {
        nm: nc.dram_tensor(nm, [H, H], BF, kind="ExternalInput").ap()
        for nm in ("wzT", "uzT", "wrT", "urT", "whT", "uhT")
    }
    bias = nc.dram_tensor("bias", [P, 3 * MB], F32, kind="ExternalInput").ap()
    outT = nc.dram_tensor("outT", [H, R], F32, kind="ExternalOutput").ap()

    with tile.TileContext(nc) as tc, ExitStack() as ctx:
        wpool = ctx.enter_context(tc.tile_pool(name="w", bufs=32))
        xpool = ctx.enter_context(tc.tile_pool(name="x", bufs=2))
        hbpool = ctx.enter_context(tc.tile_pool(name="hb", bufs=1))
        hfpool = ctx.enter_context(tc.tile_pool(name="hf", bufs=2))
        rhpool = ctx.enter_context(tc.tile_pool(name="rh", bufs=MB + 2))
        rpool = ctx.enter_context(tc.tile_pool(name="r", bufs=6))
        zpool = ctx.enter_context(tc.tile_pool(name="z", bufs=2 * MB + 2))
        gpool = ctx.enter_context(tc.tile_pool(name="g", bufs=6))
        opool = ctx.enter_context(tc.tile_pool(name="o", bufs=2))
        cpool = ctx.enter_context(tc.tile_pool(name="c", bufs=1))
        pspool = ctx.enter_context(tc.tile_pool(name="ps", bufs=8, space="PSUM"))

        # Warm up the ACT table set (sigmoid_and_others covers tanh too) on an
        # instruction with minimal sync waits — walrus can't attach the
        # PSEUDO_LOAD_ACT_FUNC_SET to an activation that already carries two
        # sem waits ("Too many sync wait commands").
        warm = cpool.tile([P, 8], F32, tag="warm")
        nc.gpsimd.memset(warm[:], 0.0)
        nc.scalar.activation(warm[:], warm[:], AF.Sigmoid)

        bt = cpool.tile([P, 3 * MB], F32, tag="bias")
        nc.sync.dma_start(bt[:], bias[:])
        # bias column layout: [z:0..7 | r:8..15 | h:16..23]
        GZ, GR, GH = 0, 1, 2

        def load_w(name, c):
            """8 k-tiles [P, H] of one weight matrix."""
            ts = []
            for k in range(KB):
                t = wpool.tile([P, H], BF, tag="w")
                nc.sync.dma_start(t[:], wd[name][k * P:(k + 1) * P, :])
                ts.append(t)
            return ts

        def mm_group(psums, wt, ut, mov_w, mov_u, m, c):
            """Accumulate  wt.T @ mov_w + ut.T @ mov_u  for feature block m
            into psums[s] ([P, NS] each), contracting over all KB blocks."""
            msl = slice(m * P, (m + 1) * P)
            for k in range(KB):
                for s in range(SL):
                    nc.tensor.matmul(
                        psums[s][:],
                        wt[k][:, msl],
                        mov_w[:, k * RC + s * NS: k * RC + (s + 1) * NS],
                        start=(k == 0), stop=False,
                    )
            for k in range(KB):
                for s in range(SL):
                    nc.tensor.matmul(
                        psums[s][:],
                        ut[k][:, msl],
                        mov_u[:, k * RC + s * NS: k * RC + (s + 1) * NS],
                        start=False, stop=(k == KB - 1),
                    )

        for c in range(CH):
            rows = slice(c * RC, (c + 1) * RC)

            # DMA emission matches the r-pass m=0 matmul consumption order
            # (wr[k] with x[k] pairs, then ur[k] with hb[k]) so the PE can
            # start as soon as the first pair lands instead of waiting for
            # the whole 8MB initial burst to drain round-robin.
            xt = xpool.tile([P, KB * RC], BF, tag="x")
            hbt = hbpool.tile([P, KB * RC], BF, tag="hb")
            wr, ur = [], []
            for k in range(KB):
                ksl = slice(k * P, (k + 1) * P)
                t = wpool.tile([P, H], BF, tag="w", name="t")
                nc.sync.dma_start(t[:], wd["wrT"][ksl, :])
                wr.append(t)
                nc.sync.dma_start(xt[:, k * RC:(k + 1) * RC], xT[ksl, rows])
            for k in range(KB):
                ksl = slice(k * P, (k + 1) * P)
                t = wpool.tile([P, H], BF, tag="w", name="t")
                nc.sync.dma_start(t[:], wd["urT"][ksl, :])
                ur.append(t)
                nc.sync.dma_start(hbt[:, k * RC:(k + 1) * RC], hTb[ksl, rows])

            # ---- r pass ----
            rhs = []
            for m in range(MB):
                ps = [pspool.tile([P, NS], F32, tag="ps", name="ps") for _ in range(SL)]
                mm_group(ps, wr, ur, xt, hbt, m, c)
                rh = rhpool.tile([P, RC], BF, tag="rh")
                for s in range(SL):
                    rt = rpool.tile([P, NS], BF, tag="r")
                    nc.scalar.activation(rt[:], ps[s][:], AF.Sigmoid,
                                         bias=bt[:, GR * MB + m: GR * MB + m + 1])
                    nc.vector.tensor_mul(
                        rh[:, s * NS:(s + 1) * NS], rt[:],
                        hbt[:, m * RC + s * NS: m * RC + (s + 1) * NS])
                rhs.append(rh)

            # ---- z pass ----
            wz = load_w("wzT", c)
            uz = load_w("uzT", c)
            zts = []
            for m in range(MB):
                ps = [pspool.tile([P, NS], F32, tag="ps", name="ps") for _ in range(SL)]
                mm_group(ps, wz, uz, xt, hbt, m, c)
                zm = []
                for s in range(SL):
                    zt = zpool.tile([P, NS], BF, tag="z")
                    nc.scalar.activation(zt[:], ps[s][:], AF.Sigmoid,
                                         bias=bt[:, GZ * MB + m: GZ * MB + m + 1])
                    zm.append(zt)
                zts.append(zm)

            # ---- h~ pass + combine ----
            wh = load_w("whT", c)
            uh = load_w("uhT", c)
            for m in range(MB):
                msl = slice(m * P, (m + 1) * P)
                hf = hfpool.tile([P, RC], F32, tag="hf")
                nc.sync.dma_start(hf[:], hTf[msl, rows])
                ps = [pspool.tile([P, NS], F32, tag="ps", name="ps") for _ in range(SL)]
                for k in range(KB):
                    for s in range(SL):
                        nc.tensor.matmul(
                            ps[s][:], wh[k][:, msl],
                            xt[:, k * RC + s * NS: k * RC + (s + 1) * NS],
                            start=(k == 0), stop=False)
                for k in range(KB):
                    for s in range(SL):
                        nc.tensor.matmul(
                            ps[s][:], uh[k][:, msl],
                            rhs[k][:, s * NS:(s + 1) * NS],
                            start=False, stop=(k == KB - 1))
                ot = opool.tile([P, RC], F32, tag="o")
                for s in range(SL):
                    ssl = slice(s * NS, (s + 1) * NS)
                    gt = gpool.tile([P, NS], F32, tag="g")
                    nc.scalar.activation(gt[:], ps[s][:], AF.Tanh,
                                         bias=bt[:, GH * MB + m: GH * MB + m + 1])
                    # g-h ; z*(g-h) ; h + z*(g-h)
                    nc.vector.tensor_sub(gt[:], gt[:], hf[:, ssl])
                    nc.vector.tensor_mul(gt[:], zts[m][s][:], gt[:])
                    nc.vector.tensor_add(ot[:, ssl], gt[:], hf[:, ssl])
                    # per-slice store so the tail DMA streams out as each
                    # slice's combine finishes instead of all at once
                    nc.sync.dma_start(
                        outT[msl, c * RC + s * NS: c * RC + (s + 1) * NS],
                        ot[:, ssl])

    nc.compile()
    return nc


_NC_CACHE = {}


def _get_nc(R, CH):
    key = (R, CH)
    if key not in _NC_CACHE:
        _NC_CACHE[key] = build_nc(R, CH)
    return _NC_CACHE[key]


def make_in_maps(update, hidden, wz, uz, bz, wr, ur, br, wh, uh, bh,
                 ncores=NCORES):
    wmap = {
        "wzT": np.ascontiguousarray(wz.T).astype(bf16),
        "uzT": np.ascontiguousarray(uz.T).astype(bf16),
        "wrT": np.ascontiguousarray(wr.T).astype(bf16),
        "urT": np.ascontiguousarray(ur.T).astype(bf16),
        "whT": np.ascontiguousarray(wh.T).astype(bf16),
        "uhT": np.ascontiguousarray(uh.T).astype(bf16),
    }
    bias = np.empty((P, 3 * MB), np.float32)
    for g, b in enumerate((bz, br, bh)):
        bias[:, g * MB:(g + 1) * MB] = np.asarray(b, np.float32).reshape(MB, P).T
    rows = update.shape[0]
    rc = rows // ncores
    in_maps = []
    for i in range(ncores):
        sl = slice(i * rc, (i + 1) * rc)
        xTs = np.ascontiguousarray(np.asarray(update[sl], np.float32).T)
        hTs = np.ascontiguousarray(np.asarray(hidden[sl], np.float32).T)
        in_maps.append(dict(xT=xTs.astype(bf16), hTb=hTs.astype(bf16),
                            hTf=hTs, bias=bias, **wmap))
    return in_maps


def kernel(update, hidden, wz, uz, bz, wr, ur, br, wh, uh, bh):
    global LAST_RESULT
    update = np.asarray(update)
    hidden = np.asarray(hidden)
    R = update.shape[0] // NCORES
    nc = _get_nc(R, 2)
    in_maps = make_in_maps(update, hidden, wz, uz, bz, wr, ur, br, wh, uh, bh)
    res = run_bass_kernel_spmd(nc, in_maps, list(range(NCORES)), trace=TRACE)
    LAST_RESULT = res
    out = np.empty((update.shape[0], H), np.float32)
    for i in range(NCORES):
        out[i * R:(i + 1) * R] = res.results[i]["outT"].T
    return out
